# revision 71
# baseline (speedup 1.0000x reference)
"""Trainium2 Bass kernel for a batched Kalman filter.

Math: the covariance/gain recursion of the Kalman filter is independent of the
measurements, and the initial covariance is identical for every batch element.
So the gain sequence K_t and transition A_t = (I - K_t H) F are batch-uniform
and computed once on the host (float64). The device then evaluates, for every
batch element b:

    x_t = A_t x_{t-1} + K_t z_t ,   output[:, t] = x_t

parallelized over time in chunks of C=8 steps. Chunk-local outputs come from
two matmuls per 128-batch half (a block-triangular L^T against the chunk's
transposed measurements, plus G^T against the chunk-entry state). The chunk
entry states themselves are NOT computed by a serial scan: the host bakes
dense prefix matrices W[k,k'] (entry state of chunk k from the measurements of
chunk k') so every entry state is a small independent matmul sum over the
transposed measurement tiles — no cross-chunk dependency chain on device.

Everything device-side is bf16 (fp32 PSUM accumulation), halving DMA traffic;
the ~2e-3 rounding error is far inside the 2e-2 tolerance.
"""

import numpy as np
import ml_dtypes

import concourse.bass as bass
import concourse.mybir as mybir
import concourse.tile as tile
from concourse.bass_utils import run_bass_kernel_spmd

S_DIM = 32
O_DIM = 16
T = 64
CH = 8            # timesteps per chunk
NCH = T // CH     # chunks
B = 2048
NCORES = 8
BS = B // NCORES  # batch per core (256)

TO = T * O_DIM            # 1024 flattened (t, o)
CHO = CH * O_DIM          # 128 per-chunk (t, o)
CHS = CH * S_DIM          # 256 per-chunk (t, s)

# Mid-depth prefix: chunks 1-2 come straight from the measurements, 3-4
# anchor on xs_2, 5-7 anchor on xs_4 — 12 measurement blocks with an anchor
# chain of depth 2 (deeper chains add serial copy latency to the tail).
def _base_of(k):
    if k <= 2:
        return 0
    if k <= 4:
        return 2
    return 4


# Entry states are packed into three [128, 256] tiles (one per anchor level)
# using PE column tiling: XTILE[k] = which SBUF/PSUM tile, XGRP[k] = which
# 32-partition group inside it. x0 (k=0) lives in its own [32, 256] tile.
# Each level's groups are contiguous so one partial copy drains the level.
XTILE = {1: 0, 2: 0, 3: 1, 4: 1, 5: 2, 6: 2, 7: 3}
XGRP = {0: 0, 1: 2, 2: 3, 3: 2, 4: 3, 5: 0, 6: 1, 7: 2}
# partition range each tile's PSUM->SBUF copy drains; DVE bank routing only
# allows 64-partition moves at base 0/64 and <=32-partition moves anywhere
XCOPY = {0: (64, 128), 1: (64, 128), 2: (0, 64), 3: (64, 96)}
XTILES = ((1, 2), (3, 4), (5, 6), (7,))

# c32 constants live on all 128 partitions: block (gpt_k / gt_k) is stored at
# partition group g with column offset c inside a [128, 768] tile.
# gpt_k must sit in the partition group of its anchor (so the stationary
# loads into the array rows its streamed anchor feeds); gt_k in XGRP[k].
GPT_POS = {1: (0, 0), 2: (0, 32),                # anchor x0t (group 0)
           3: (3, 0), 4: (3, 32),                # anchor xs_2 (group 3)
           5: (3, 64), 6: (3, 96), 7: (3, 128)}  # anchor xs_4 (group 3)
GT_POS = {0: (0, 64), 5: (0, 320),
          6: (1, 0),
          1: (2, 0), 3: (2, 256), 7: (2, 512),
          2: (3, 160), 4: (3, 416)}
C32P_COLS = 768


W_IDX = {}
for _k in range(1, NCH):
    for _kp in range(_base_of(_k), _k):
        W_IDX[(_k, _kp)] = len(W_IDX)
NW = len(W_IDX)  # 10 dense prefix blocks

BF16 = mybir.dt.bfloat16
F32 = mybir.dt.float32
NPBF16 = ml_dtypes.bfloat16

# column offsets inside the small 128-partition const tile: ident | wt
C_WT = 128
CW_COLS = C_WT + NW * S_DIM      # 128 + 384


def _host_mats(F, H, Q, R, P0):
    """Batch-uniform Kalman gain/transition derived matrices, in float64."""
    I = np.eye(S_DIM)
    P = P0
    A_list, K_list = [], []
    for _ in range(T):
        P_pred = F @ P @ F.T + Q
        S = H @ P_pred @ H.T + R
        K = P_pred @ H.T @ np.linalg.inv(S)
        A = (I - K @ H) @ F
        P = (I - K @ H) @ P_pred
        A_list.append(A)
        K_list.append(K)

    # Per-chunk output operators:
    #   G[k, i] : chunk-entry state -> state at step i of chunk k
    #   L[k, i, j] : measurements at step j of chunk k -> state at step i
    G = np.zeros((NCH, CH, S_DIM, S_DIM))
    L = np.zeros((NCH, CH, CH, S_DIM, O_DIM))
    for k in range(NCH):
        for i in range(CH):
            t = CH * k + i
            G[k, i] = A_list[t] @ (G[k, i - 1] if i > 0 else I)
            for j in range(i):
                L[k, i, j] = A_list[t] @ L[k, i - 1, j]
            L[k, i, i] = K_list[t]

    # gt[s', k, i*32+s] = G[k, i, s, s']   (32, NCH, CHS)
    gt = np.ascontiguousarray(G.transpose(3, 0, 1, 2).reshape(S_DIM, NCH, CHS))
    # lt[j*16+o, k, i*32+s] = L[k, i, j, s, o]   (CHO, NCH, CHS)
    lt = np.ascontiguousarray(
        L.transpose(2, 4, 0, 1, 3).reshape(CHO, NCH, CHS))

    # Multi-level dense prefix operators for chunk-entry states:
    #   xs_k = GP_k xs_base + sum_{base<=k'<k} W[k,k'] zc_{k'},  base=_base_of(k)
    # with Gc_m = G[m, CH-1], Lc_m[s,(j,o)] = L[m, CH-1, j][s,o].
    Gc = [G[m, CH - 1] for m in range(NCH)]
    Lc = [np.ascontiguousarray(
        L[m, CH - 1].transpose(1, 0, 2).reshape(S_DIM, CHO))
        for m in range(NCH)]

    def Mprod(k, k0):  # Gc_{k-1} @ ... @ Gc_{k0}; identity if k0 == k
        Mp = np.eye(S_DIM)
        for m in range(k0, k):
            Mp = Gc[m] @ Mp
        return Mp

    # gpt[s_in, k-1, s_out] = GP_k[s_out, s_in]   (32, NCH-1, 32)
    gpt = np.zeros((S_DIM, NCH - 1, S_DIM))
    # wt[(j,o), idx, s_out] = W[k,k'][s_out, (j,o)]   (CHO, NW, 32)
    wt = np.zeros((CHO, NW, S_DIM))
    for k in range(1, NCH):
        base = _base_of(k)
        gpt[:, k - 1, :] = Mprod(k, base).T
        for kp in range(base, k):
            idx = W_IDX[(k, kp)]
            wt[:, idx, :] = (Mprod(k, kp + 1) @ Lc[kp]).T

    cw = np.zeros((128, CW_COLS), dtype=np.float64)
    cw[:, :128] = np.eye(128)
    cw[:CHO, C_WT:] = wt.reshape(CHO, NW * S_DIM)
    c32 = np.zeros((128, C32P_COLS), dtype=np.float64)
    for k in range(1, NCH):
        g, c = GPT_POS[k]
        c32[32 * g:32 * (g + 1), c:c + S_DIM] = gpt[:, k - 1, :]
    for k in range(NCH):
        g, c = GT_POS[k]
        c32[32 * g:32 * (g + 1), c:c + CHS] = gt[:, k, :]
    ltf = lt.reshape(CHO, NCH * CHS)
    return (cw.astype(NPBF16), ltf.astype(NPBF16), c32.astype(NPBF16))


NFLUSH = T * S_DIM            # 2048 bf16 output columns


def build_nc(reps=1):
    nc = bass.Bass("TRN2", target_bir_lowering=False, debug=False,
                   num_devices=1)

    z_d = nc.dram_tensor("z", (BS, TO + S_DIM), BF16, kind="ExternalInput")
    cw_d = nc.dram_tensor("cw", (128, CW_COLS), BF16, kind="ExternalInput")
    lt_d = nc.dram_tensor("lt", (CHO, NCH * CHS), BF16, kind="ExternalInput")
    c32_d = nc.dram_tensor("c32", (128, C32P_COLS), BF16, kind="ExternalInput")
    out_d = nc.dram_tensor("out", (BS, NFLUSH), BF16, kind="ExternalOutput")

    with tile.TileContext(nc) as tc:
        with (
            tc.tile_pool(name="const", bufs=1) as const,
            tc.tile_pool(name="zin", bufs=1) as zin_p,
            tc.tile_pool(name="zt", bufs=1) as zt_p,
            tc.tile_pool(name="xs", bufs=1) as xs_p,
            tc.tile_pool(name="outb", bufs=1) as outb_p,
            tc.tile_pool(name="pszt", bufs=2, space="PSUM") as ps_zt,
            tc.tile_pool(name="psxs", bufs=2, space="PSUM") as ps_xs,
            tc.tile_pool(name="psout", bufs=3, space="PSUM") as ps_out,
        ):
          for _rep in range(reps):
            # warm the Activation engine's function table during the input
            # DMAs so the first real scalar.copy doesn't stall ~1.3us; dtypes
            # must match the real copies (f32 -> bf16) to hit the same table
            warm = const.tile([1, 2], F32, name="warm")
            nc.vector.memset(warm[:], 0)
            warm2 = const.tile([1, 2], BF16, name="warm2")
            nc.scalar.copy(warm2[:], warm[:])

            # input DMAs spread over the three issue queues (SP / Act / Pool)
            # so the transfers overlap; each queue's first transfer carries
            # what unblocks the PE earliest (z halves + identity/prefix)
            zin = []
            zi0 = zin_p.tile([128, TO + S_DIM], BF16, name="zin0")
            nc.sync.dma_start(zi0[:], z_d[0:128])
            zin.append(zi0)
            zi1 = zin_p.tile([128, TO + S_DIM], BF16, name="zin1")
            nc.scalar.dma_start(zi1[:], z_d[128:256])
            zin.append(zi1)
            cw = const.tile([128, CW_COLS], BF16)
            nc.gpsimd.dma_start(cw[:], cw_d[:])
            c32 = const.tile([128, C32P_COLS], BF16)
            nc.gpsimd.dma_start(c32[:], c32_d[:])
            ltt = const.tile([CHO, NCH * CHS], BF16)
            q_lt = NCH * CHS // 4
            for q in range(4):
                eng = nc.sync if q < 2 else nc.scalar
                eng.dma_start(ltt[:, q * q_lt:(q + 1) * q_lt],
                              lt_d[:, q * q_lt:(q + 1) * q_lt])
            ident = cw[:, :128]

            def lt_k(k):
                return ltt[:, k * CHS:(k + 1) * CHS]

            def wt_idx(idx):
                return cw[:CHO, C_WT + idx * S_DIM:C_WT + (idx + 1) * S_DIM]

            def gpt_k(k):
                g, c = GPT_POS[k]
                return c32[32 * g:32 * (g + 1), c:c + S_DIM]

            def gt_k(k):
                g, c = GT_POS[k]
                return c32[32 * g:32 * (g + 1), c:c + CHS]

            # x0 transposed -> (32, 256) bf16. Transposes are spelled as
            # REGULAR matmuls against a streamed identity (out = z.T @ I):
            # same math as transpose-mode but avoids its fixed SBUF-access
            # latency and keeps the PE clock-gate (HAM) warm.
            xt_ps = ps_xs.tile([S_DIM, 256], F32, name="xtps", bufs=1)
            for h in range(2):
                nc.tensor.matmul(
                    xt_ps[:, h * 128:(h + 1) * 128],
                    zin[h][:, TO:TO + S_DIM], ident)
            x0t = xs_p.tile([S_DIM, 256], BF16, name="x0t")
            nc.vector.tensor_copy(x0t[:], xt_ps[:])

            # measurements transposed per chunk: zt[k] is (128 = 8t x 16o, 256b)
            ztiles = []
            for k in range(NCH):
                zt_ps = ps_zt.tile([128, 256], F32, name="ztps")
                for h in range(2):
                    nc.tensor.matmul(
                        zt_ps[:, h * 128:(h + 1) * 128],
                        zin[h][:, k * CHO:(k + 1) * CHO], ident)
                zt_sb = zt_p.tile([128, 256], BF16, name=f"zt{k}")
                if k % 2 == 0:
                    nc.vector.tensor_copy(zt_sb[:], zt_ps[:])
                else:
                    nc.scalar.copy(zt_sb[:], zt_ps[:])
                ztiles.append(zt_sb)

            # chunk-entry states, packed 2-3 per [128, 256] tile via PE
            # column tiling (XGRP picks the 32-partition group). One copy
            # drains each anchor level instead of one per chunk, and on HW
            # the small matmuls of one level run concurrently in the array.
            xs_tiles = []

            def xs_slice(k, h=None):
                if k == 0:
                    src = x0t
                    g = 0
                else:
                    src = xs_tiles[XTILE[k]]
                    g = XGRP[k]
                cols = src[32 * g:32 * (g + 1), :] if k else src[:, :]
                if h is None:
                    return cols
                return cols[:, h * 128:(h + 1) * 128]

            for tl, ks in enumerate(XTILES):
                c_ps = ps_xs.tile([128, 256], F32, name="cps")
                for k in ks:
                    base = _base_of(k)
                    g = XGRP[k]
                    out_sl = c_ps[32 * g:32 * (g + 1), :]
                    for kp in range(base, k):
                        idx = W_IDX[(k, kp)]
                        nc.tensor.matmul(out_sl, wt_idx(idx), ztiles[kp][:],
                                         start=(kp == base), stop=False,
                                         tile_position=(0, 32 * g))
                    ganc = 0 if base == 0 else XGRP[base]
                    nc.tensor.matmul(out_sl, gpt_k(k), xs_slice(base),
                                     start=False, stop=True,
                                     tile_position=(32 * ganc, 32 * g))
                xs_sb = xs_p.tile([128, 256], BF16, name=f"xsp{tl}")
                lo, hi = XCOPY[tl]
                if tl % 2 == 0:
                    nc.vector.tensor_copy(xs_sb[lo:hi, :], c_ps[lo:hi, :])
                else:
                    nc.scalar.copy(xs_sb[lo:hi, :], c_ps[lo:hi, :])
                xs_tiles.append(xs_sb)

            # per-chunk outputs: two matmuls per (chunk, batch-half), pairs of
            # chunks share one PSUM bank so one copy drains 512 columns.
            # Each (flush-group, half) gets its OWN SBUF tile so a flush DMA
            # depends only on the copies of its group, not all of outb.
            groups = {0: (0, 1024), 1: (1024, 1536), 2: (1536, 2048)}
            grp_of = {0: 0, 1: 0, 2: 1, 3: 2}
            outb = {(g, h): outb_p.tile([128, hi - lo], BF16, name=f"ob{g}{h}")
                    for g, (lo, hi) in groups.items() for h in range(2)}
            for p in range(NCH // 2):
                g = grp_of[p]
                lo, hi = groups[g]
                for h in range(2):
                    o_ps = ps_out.tile([128, 2 * CHS], F32, name="ops")
                    for q in range(2):
                        k = 2 * p + q
                        sl = o_ps[:, q * CHS:(q + 1) * CHS]
                        nc.tensor.matmul(
                            sl, ztiles[k][:, h * 128:(h + 1) * 128], lt_k(k),
                            start=True, stop=False)
                        nc.tensor.matmul(
                            sl, xs_slice(k, h), gt_k(k),
                            start=False, stop=True,
                            tile_position=(32 * XGRP[k], 0))
                    off = p * 2 * CHS - lo
                    dst = outb[(g, h)][:, off:off + 2 * CHS]
                    if h == 0:
                        nc.vector.tensor_copy(dst, o_ps[:])
                    else:
                        nc.scalar.copy(dst, o_ps[:])
                if p in (1, 2, 3):
                    g = grp_of[p]
                    lo, hi = groups[g]
                    for h in range(2):
                        eng = nc.sync if h == 0 else nc.gpsimd
                        eng.dma_start(out_d[h * 128:(h + 1) * 128, lo:hi],
                                      outb[(g, h)][:])

    _split_matmul_waits(nc)
    return nc


def _split_matmul_waits(nc, max_waits=1):
    """Walrus lowers matmuls through a template with fewer sync-wait slots
    than Tile may emit. Move excess waits onto a PE NoOp inserted right
    before the offending instruction."""
    for f in nc.m.functions:
        for blk in f.blocks:
            insts = list(blk.instructions)
            out = []
            for inst in insts:
                si = inst.sync_info
                if si is not None and si.on_wait and len(si.on_wait) > max_waits:
                    waits = list(si.on_wait)
                    carry, keep = waits[:-max_waits], waits[-max_waits:]
                    for w in carry:
                        nop = mybir.InstNoOp(
                            name=nc.get_next_instruction_name(),
                            sync_info=mybir.SyncInfo(on_wait=[w], on_update=[]),
                            bass_nofuse=True,
                            engine=inst.engine,
                        )
                        out.append(nop)
                    inst.sync_info = mybir.SyncInfo(
                        on_wait=keep, on_update=list(si.on_update or [])
                    )
                out.append(inst)
            if len(out) != len(insts):
                blk.instructions = out


def _sim_feeds(inputs):
    """Per-core-0 input feeds for CoreSim-based analysis (test.py/analyze.py)."""
    cw, ltf, c32 = _host_mats(
        np.asarray(inputs["F"], np.float64), np.asarray(inputs["H"], np.float64),
        np.asarray(inputs["Q"], np.float64), np.asarray(inputs["R"], np.float64),
        np.asarray(inputs["cov0"], np.float64)[0])
    zcat = np.concatenate(
        [np.asarray(inputs["measurements"], np.float32).reshape(B, TO),
         np.asarray(inputs["state0"], np.float32)], axis=1).astype(NPBF16)
    return {"z": zcat[:BS], "cw": cw, "lt": ltf, "c32": c32}


_CACHE = {}


def kernel(state0, cov0, measurements, F, H, Q, R, _trace=False):
    state0 = np.asarray(state0, np.float32)
    measurements = np.asarray(measurements, np.float32)
    cw, ltf, c32 = _host_mats(
        np.asarray(F, np.float64), np.asarray(H, np.float64),
        np.asarray(Q, np.float64), np.asarray(R, np.float64),
        np.asarray(cov0, np.float64)[0],
    )
    zcat = np.concatenate(
        [measurements.reshape(B, TO), state0], axis=1).astype(NPBF16)

    if "nc" not in _CACHE:
        _CACHE["nc"] = build_nc()
    nc = _CACHE["nc"]

    in_maps = [
        {"z": zcat[c * BS:(c + 1) * BS], "cw": cw, "lt": ltf, "c32": c32}
        for c in range(NCORES)
    ]
    res = run_bass_kernel_spmd(nc, in_maps, core_ids=list(range(NCORES)),
                               trace=_trace)
    out = np.concatenate(
        [np.asarray(res.results[c]["out"]) for c in range(NCORES)], axis=0)
    if _trace:
        kernel._last_result = res
    return out.astype(np.float32).reshape(B, T, S_DIM)


# revision 72
# speedup vs baseline: 1.0228x; 1.0228x over previous
"""Trainium2 Bass kernel for a batched Kalman filter.

Math: the covariance/gain recursion of the Kalman filter is independent of the
measurements, and the initial covariance is identical for every batch element.
So the gain sequence K_t and transition A_t = (I - K_t H) F are batch-uniform
and computed once on the host (float64). The device then evaluates, for every
batch element b:

    x_t = A_t x_{t-1} + K_t z_t ,   output[:, t] = x_t

parallelized over time in chunks of C=8 steps. Chunk-local outputs come from
two matmuls per 128-batch half (a block-triangular L^T against the chunk's
transposed measurements, plus G^T against the chunk-entry state). The chunk
entry states themselves are NOT computed by a serial scan: the host bakes
dense prefix matrices W[k,k'] (entry state of chunk k from the measurements of
chunk k') so every entry state is a small independent matmul sum over the
transposed measurement tiles — no cross-chunk dependency chain on device.

Everything device-side is bf16 (fp32 PSUM accumulation), halving DMA traffic;
the ~2e-3 rounding error is far inside the 2e-2 tolerance.
"""

import numpy as np
import ml_dtypes

import concourse.bass as bass
import concourse.mybir as mybir
import concourse.tile as tile
from concourse.bass_utils import run_bass_kernel_spmd

S_DIM = 32
O_DIM = 16
T = 64
CH = 8            # timesteps per chunk
NCH = T // CH     # chunks
B = 2048
NCORES = 8
BS = B // NCORES  # batch per core (256)

TO = T * O_DIM            # 1024 flattened (t, o)
CHO = CH * O_DIM          # 128 per-chunk (t, o)
CHS = CH * S_DIM          # 256 per-chunk (t, s)

# Mid-depth prefix: chunks 1-2 come straight from the measurements, 3-4
# anchor on xs_2, 5-7 anchor on xs_4 — 12 measurement blocks with an anchor
# chain of depth 2 (deeper chains add serial copy latency to the tail).
def _base_of(k):
    if k <= 2:
        return 0
    if k <= 4:
        return 2
    return 4


# Entry states are packed into three [128, 256] tiles (one per anchor level)
# using PE column tiling: XTILE[k] = which SBUF/PSUM tile, XGRP[k] = which
# 32-partition group inside it. x0 (k=0) lives in its own [32, 256] tile.
# Each level's groups are contiguous so one partial copy drains the level.
XTILE = {1: 0, 2: 0, 3: 1, 4: 1, 5: 2, 6: 2, 7: 3}
XGRP = {0: 0, 1: 2, 2: 3, 3: 2, 4: 3, 5: 0, 6: 1, 7: 2}
# partition range each tile's PSUM->SBUF copy drains; DVE bank routing only
# allows 64-partition moves at base 0/64 and <=32-partition moves anywhere
XCOPY = {0: (64, 128), 1: (64, 128), 2: (0, 64), 3: (64, 96)}
XTILES = ((1, 2), (3, 4), (5, 6), (7,))

# c32 constants live on all 128 partitions: block (gpt_k / gt_k) is stored at
# partition group g with column offset c inside a [128, 768] tile.
# gpt_k must sit in the partition group of its anchor (so the stationary
# loads into the array rows its streamed anchor feeds); gt_k in XGRP[k].
GPT_POS = {1: (0, 0), 2: (0, 32),                # anchor x0t (group 0)
           3: (3, 0), 4: (3, 32),                # anchor xs_2 (group 3)
           5: (3, 64), 6: (3, 96), 7: (3, 128)}  # anchor xs_4 (group 3)
GT_POS = {0: (0, 64), 5: (0, 320),
          6: (1, 0),
          1: (2, 0), 3: (2, 256), 7: (2, 512),
          2: (3, 160), 4: (3, 416)}
C32P_COLS = 768


W_IDX = {}
for _k in range(1, NCH):
    for _kp in range(_base_of(_k), _k):
        W_IDX[(_k, _kp)] = len(W_IDX)
NW = len(W_IDX)  # 10 dense prefix blocks

BF16 = mybir.dt.bfloat16
F32 = mybir.dt.float32
NPBF16 = ml_dtypes.bfloat16

# column offsets inside the small 128-partition const tile: ident | wt
C_WT = 128
CW_COLS = C_WT + NW * S_DIM      # 128 + 384


def _host_mats(F, H, Q, R, P0):
    """Batch-uniform Kalman gain/transition derived matrices, in float64."""
    I = np.eye(S_DIM)
    P = P0
    A_list, K_list = [], []
    for _ in range(T):
        P_pred = F @ P @ F.T + Q
        S = H @ P_pred @ H.T + R
        K = P_pred @ H.T @ np.linalg.inv(S)
        A = (I - K @ H) @ F
        P = (I - K @ H) @ P_pred
        A_list.append(A)
        K_list.append(K)

    # Per-chunk output operators:
    #   G[k, i] : chunk-entry state -> state at step i of chunk k
    #   L[k, i, j] : measurements at step j of chunk k -> state at step i
    G = np.zeros((NCH, CH, S_DIM, S_DIM))
    L = np.zeros((NCH, CH, CH, S_DIM, O_DIM))
    for k in range(NCH):
        for i in range(CH):
            t = CH * k + i
            G[k, i] = A_list[t] @ (G[k, i - 1] if i > 0 else I)
            for j in range(i):
                L[k, i, j] = A_list[t] @ L[k, i - 1, j]
            L[k, i, i] = K_list[t]

    # gt[s', k, i*32+s] = G[k, i, s, s']   (32, NCH, CHS)
    gt = np.ascontiguousarray(G.transpose(3, 0, 1, 2).reshape(S_DIM, NCH, CHS))
    # lt[j*16+o, k, i*32+s] = L[k, i, j, s, o]   (CHO, NCH, CHS)
    lt = np.ascontiguousarray(
        L.transpose(2, 4, 0, 1, 3).reshape(CHO, NCH, CHS))

    # Multi-level dense prefix operators for chunk-entry states:
    #   xs_k = GP_k xs_base + sum_{base<=k'<k} W[k,k'] zc_{k'},  base=_base_of(k)
    # with Gc_m = G[m, CH-1], Lc_m[s,(j,o)] = L[m, CH-1, j][s,o].
    Gc = [G[m, CH - 1] for m in range(NCH)]
    Lc = [np.ascontiguousarray(
        L[m, CH - 1].transpose(1, 0, 2).reshape(S_DIM, CHO))
        for m in range(NCH)]

    def Mprod(k, k0):  # Gc_{k-1} @ ... @ Gc_{k0}; identity if k0 == k
        Mp = np.eye(S_DIM)
        for m in range(k0, k):
            Mp = Gc[m] @ Mp
        return Mp

    # gpt[s_in, k-1, s_out] = GP_k[s_out, s_in]   (32, NCH-1, 32)
    gpt = np.zeros((S_DIM, NCH - 1, S_DIM))
    # wt[(j,o), idx, s_out] = W[k,k'][s_out, (j,o)]   (CHO, NW, 32)
    wt = np.zeros((CHO, NW, S_DIM))
    for k in range(1, NCH):
        base = _base_of(k)
        gpt[:, k - 1, :] = Mprod(k, base).T
        for kp in range(base, k):
            idx = W_IDX[(k, kp)]
            wt[:, idx, :] = (Mprod(k, kp + 1) @ Lc[kp]).T

    cw = np.zeros((128, CW_COLS), dtype=np.float64)
    cw[:, :128] = np.eye(128)
    cw[:CHO, C_WT:] = wt.reshape(CHO, NW * S_DIM)
    c32 = np.zeros((128, C32P_COLS), dtype=np.float64)
    for k in range(1, NCH):
        g, c = GPT_POS[k]
        c32[32 * g:32 * (g + 1), c:c + S_DIM] = gpt[:, k - 1, :]
    for k in range(NCH):
        g, c = GT_POS[k]
        c32[32 * g:32 * (g + 1), c:c + CHS] = gt[:, k, :]
    ltf = lt.reshape(CHO, NCH * CHS)
    return (cw.astype(NPBF16), ltf.astype(NPBF16), c32.astype(NPBF16))


NFLUSH = T * S_DIM            # 2048 bf16 output columns


def build_nc(reps=1):
    nc = bass.Bass("TRN2", target_bir_lowering=False, debug=False,
                   num_devices=NCORES)

    z_d = nc.dram_tensor("z", (BS, TO + S_DIM), BF16, kind="ExternalInput")
    cw_d = nc.dram_tensor("cw", (128, CW_COLS), BF16, kind="ExternalInput")
    lt_d = nc.dram_tensor("lt", (CHO, NCH * CHS), BF16, kind="ExternalInput")
    c32_d = nc.dram_tensor("c32", (128, C32P_COLS), BF16, kind="ExternalInput")
    out_d = nc.dram_tensor("out", (BS, NFLUSH), BF16, kind="ExternalOutput")

    with tile.TileContext(nc) as tc:
        with (
            tc.tile_pool(name="const", bufs=1) as const,
            tc.tile_pool(name="zin", bufs=1) as zin_p,
            tc.tile_pool(name="zt", bufs=1) as zt_p,
            tc.tile_pool(name="xs", bufs=1) as xs_p,
            tc.tile_pool(name="outb", bufs=1) as outb_p,
            tc.tile_pool(name="pszt", bufs=2, space="PSUM") as ps_zt,
            tc.tile_pool(name="psxs", bufs=2, space="PSUM") as ps_xs,
            tc.tile_pool(name="psout", bufs=3, space="PSUM") as ps_out,
        ):
          for _rep in range(reps):
            # warm the Activation engine's function table during the input
            # DMAs so the first real scalar.copy doesn't stall ~1.3us; dtypes
            # must match the real copies (f32 -> bf16) to hit the same table
            warm = const.tile([1, 2], F32, name="warm")
            nc.vector.memset(warm[:], 0)
            warm2 = const.tile([1, 2], BF16, name="warm2")
            nc.scalar.copy(warm2[:], warm[:])

            # input DMAs spread over the three issue queues (SP / Act / Pool)
            # so the transfers overlap; each queue's first transfer carries
            # what unblocks the PE earliest (z halves + identity/prefix)
            zin = []
            zi0 = zin_p.tile([128, TO + S_DIM], BF16, name="zin0")
            nc.sync.dma_start(zi0[:], z_d[0:128])
            zin.append(zi0)
            zi1 = zin_p.tile([128, TO + S_DIM], BF16, name="zin1")
            nc.scalar.dma_start(zi1[:], z_d[128:256])
            zin.append(zi1)
            cw = const.tile([128, CW_COLS], BF16)
            nc.gpsimd.dma_start(cw[:], cw_d[:])
            c32 = const.tile([128, C32P_COLS], BF16)
            nc.gpsimd.dma_start(c32[:], c32_d[:])
            ltt = const.tile([CHO, NCH * CHS], BF16)
            q_lt = NCH * CHS // 4
            for q in range(4):
                eng = nc.sync if q < 2 else nc.scalar
                eng.dma_start(ltt[:, q * q_lt:(q + 1) * q_lt],
                              lt_d[:, q * q_lt:(q + 1) * q_lt])
            ident = cw[:, :128]

            def lt_k(k):
                return ltt[:, k * CHS:(k + 1) * CHS]

            def wt_idx(idx):
                return cw[:CHO, C_WT + idx * S_DIM:C_WT + (idx + 1) * S_DIM]

            def gpt_k(k):
                g, c = GPT_POS[k]
                return c32[32 * g:32 * (g + 1), c:c + S_DIM]

            def gt_k(k):
                g, c = GT_POS[k]
                return c32[32 * g:32 * (g + 1), c:c + CHS]

            # x0 transposed -> (32, 256) bf16. Transposes are spelled as
            # REGULAR matmuls against a streamed identity (out = z.T @ I):
            # same math as transpose-mode but avoids its fixed SBUF-access
            # latency and keeps the PE clock-gate (HAM) warm.
            xt_ps = ps_xs.tile([S_DIM, 256], F32, name="xtps", bufs=1)
            for h in range(2):
                nc.tensor.matmul(
                    xt_ps[:, h * 128:(h + 1) * 128],
                    zin[h][:, TO:TO + S_DIM], ident)
            x0t = xs_p.tile([S_DIM, 256], BF16, name="x0t")
            nc.vector.tensor_copy(x0t[:], xt_ps[:])

            # measurements transposed per chunk: zt[k] is (128 = 8t x 16o, 256b)
            ztiles = []
            for k in range(NCH):
                zt_ps = ps_zt.tile([128, 256], F32, name="ztps")
                for h in range(2):
                    nc.tensor.matmul(
                        zt_ps[:, h * 128:(h + 1) * 128],
                        zin[h][:, k * CHO:(k + 1) * CHO], ident)
                zt_sb = zt_p.tile([128, 256], BF16, name=f"zt{k}")
                if k % 2 == 0:
                    nc.vector.tensor_copy(zt_sb[:], zt_ps[:])
                else:
                    nc.scalar.copy(zt_sb[:], zt_ps[:])
                ztiles.append(zt_sb)

            # chunk-entry states, packed 2-3 per [128, 256] tile via PE
            # column tiling (XGRP picks the 32-partition group). One copy
            # drains each anchor level instead of one per chunk, and on HW
            # the small matmuls of one level run concurrently in the array.
            xs_tiles = []

            def xs_slice(k, h=None):
                if k == 0:
                    src = x0t
                    g = 0
                else:
                    src = xs_tiles[XTILE[k]]
                    g = XGRP[k]
                cols = src[32 * g:32 * (g + 1), :] if k else src[:, :]
                if h is None:
                    return cols
                return cols[:, h * 128:(h + 1) * 128]

            for tl, ks in enumerate(XTILES):
                c_ps = ps_xs.tile([128, 256], F32, name="cps")
                for k in ks:
                    base = _base_of(k)
                    g = XGRP[k]
                    out_sl = c_ps[32 * g:32 * (g + 1), :]
                    for kp in range(base, k):
                        idx = W_IDX[(k, kp)]
                        nc.tensor.matmul(out_sl, wt_idx(idx), ztiles[kp][:],
                                         start=(kp == base), stop=False,
                                         tile_position=(0, 32 * g))
                    ganc = 0 if base == 0 else XGRP[base]
                    nc.tensor.matmul(out_sl, gpt_k(k), xs_slice(base),
                                     start=False, stop=True,
                                     tile_position=(32 * ganc, 32 * g))
                xs_sb = xs_p.tile([128, 256], BF16, name=f"xsp{tl}")
                lo, hi = XCOPY[tl]
                if tl % 2 == 0:
                    nc.vector.tensor_copy(xs_sb[lo:hi, :], c_ps[lo:hi, :])
                else:
                    nc.scalar.copy(xs_sb[lo:hi, :], c_ps[lo:hi, :])
                xs_tiles.append(xs_sb)

            # per-chunk outputs: two matmuls per (chunk, batch-half), pairs of
            # chunks share one PSUM bank so one copy drains 512 columns.
            # Each (flush-group, half) gets its OWN SBUF tile so a flush DMA
            # depends only on the copies of its group, not all of outb.
            groups = {0: (0, 1024), 1: (1024, 1536), 2: (1536, 2048)}
            grp_of = {0: 0, 1: 0, 2: 1, 3: 2}
            outb = {(g, h): outb_p.tile([128, hi - lo], BF16, name=f"ob{g}{h}")
                    for g, (lo, hi) in groups.items() for h in range(2)}
            for p in range(NCH // 2):
                g = grp_of[p]
                lo, hi = groups[g]
                for h in range(2):
                    o_ps = ps_out.tile([128, 2 * CHS], F32, name="ops")
                    for q in range(2):
                        k = 2 * p + q
                        sl = o_ps[:, q * CHS:(q + 1) * CHS]
                        nc.tensor.matmul(
                            sl, ztiles[k][:, h * 128:(h + 1) * 128], lt_k(k),
                            start=True, stop=False)
                        nc.tensor.matmul(
                            sl, xs_slice(k, h), gt_k(k),
                            start=False, stop=True,
                            tile_position=(32 * XGRP[k], 0))
                    off = p * 2 * CHS - lo
                    dst = outb[(g, h)][:, off:off + 2 * CHS]
                    if h == 0:
                        nc.vector.tensor_copy(dst, o_ps[:])
                    else:
                        nc.scalar.copy(dst, o_ps[:])
                if p in (1, 2, 3):
                    g = grp_of[p]
                    lo, hi = groups[g]
                    for h in range(2):
                        eng = nc.sync if h == 0 else nc.gpsimd
                        eng.dma_start(out_d[h * 128:(h + 1) * 128, lo:hi],
                                      outb[(g, h)][:])

    _split_matmul_waits(nc)
    return nc


def _split_matmul_waits(nc, max_waits=1):
    """Walrus lowers matmuls through a template with fewer sync-wait slots
    than Tile may emit. Move excess waits onto a PE NoOp inserted right
    before the offending instruction."""
    for f in nc.m.functions:
        for blk in f.blocks:
            insts = list(blk.instructions)
            out = []
            for inst in insts:
                si = inst.sync_info
                if si is not None and si.on_wait and len(si.on_wait) > max_waits:
                    waits = list(si.on_wait)
                    carry, keep = waits[:-max_waits], waits[-max_waits:]
                    for w in carry:
                        nop = mybir.InstNoOp(
                            name=nc.get_next_instruction_name(),
                            sync_info=mybir.SyncInfo(on_wait=[w], on_update=[]),
                            bass_nofuse=True,
                            engine=inst.engine,
                        )
                        out.append(nop)
                    inst.sync_info = mybir.SyncInfo(
                        on_wait=keep, on_update=list(si.on_update or [])
                    )
                out.append(inst)
            if len(out) != len(insts):
                blk.instructions = out


def _sim_feeds(inputs):
    """Per-core-0 input feeds for CoreSim-based analysis (test.py/analyze.py)."""
    cw, ltf, c32 = _host_mats(
        np.asarray(inputs["F"], np.float64), np.asarray(inputs["H"], np.float64),
        np.asarray(inputs["Q"], np.float64), np.asarray(inputs["R"], np.float64),
        np.asarray(inputs["cov0"], np.float64)[0])
    zcat = np.concatenate(
        [np.asarray(inputs["measurements"], np.float32).reshape(B, TO),
         np.asarray(inputs["state0"], np.float32)], axis=1).astype(NPBF16)
    return {"z": zcat[:BS], "cw": cw, "lt": ltf, "c32": c32}


_CACHE = {}


def kernel(state0, cov0, measurements, F, H, Q, R, _trace=False):
    state0 = np.asarray(state0, np.float32)
    measurements = np.asarray(measurements, np.float32)
    cw, ltf, c32 = _host_mats(
        np.asarray(F, np.float64), np.asarray(H, np.float64),
        np.asarray(Q, np.float64), np.asarray(R, np.float64),
        np.asarray(cov0, np.float64)[0],
    )
    zcat = np.concatenate(
        [measurements.reshape(B, TO), state0], axis=1).astype(NPBF16)

    if "nc" not in _CACHE:
        _CACHE["nc"] = build_nc()
    nc = _CACHE["nc"]

    in_maps = [
        {"z": zcat[c * BS:(c + 1) * BS], "cw": cw, "lt": ltf, "c32": c32}
        for c in range(NCORES)
    ]
    res = run_bass_kernel_spmd(nc, in_maps, core_ids=list(range(NCORES)),
                               trace=_trace)
    out = np.concatenate(
        [np.asarray(res.results[c]["out"]) for c in range(NCORES)], axis=0)
    if _trace:
        kernel._last_result = res
    return out.astype(np.float32).reshape(B, T, S_DIM)
